# revision 9
# baseline (speedup 1.0000x reference)
"""Multi-head causal attention (RoPE) forward on 8 Trainium2 NeuronCores.

Sharding: tensor-parallel over heads -- 8 cores x 2 heads, each core handling
both batch elements (the flattened (B*T) = 4096 "time" axis).

v2: all SBUF-resident data and the all-to-all payload are bfloat16 (PE speed
is the same 1 col/cycle as float32r, but HBM traffic, collective bytes and
SBUF pressure halve; DVE gets 2x mode). Startup DMA descriptor generation is
spread across the scalar/vector/gpsimd queues so the first xt chunk is not
serialized behind ~10 MB of weight loads on the sync queue. Attention output
leaves the core UNnormalized with the softmax denominators as a 129th row of
the a2a payload; normalization happens after the a2a (one reciprocal per
source instead of 16, and the phase-2 critical path drops the
recip/broadcast/mul chain). Phase 3 runs in two passes -- the 8 matmul
contributions of a2a0 are accumulated and parked in SBUF while a2a1 is still
in flight, then pass B adds the rest; wo is prefetched in full (8 MB bf16)
during phase 2.

Per core:
  phase 1 (TC1=512 chunks): qT/kT [d, B*T] and v [B*T, d] projections from
           host-pre-transposed xT, RoPE via a +-1 pair-swap permutation
           matmul on PE plus elementwise combine with interleaved cos/sin.
  phase 2: per (head, batch), scores^T [j, i] = kT^T @ qT, exp on ScalarE
           (no max pass -- bounded score distribution), mask as additive
           bias on partially-masked tiles only, fully-masked tiles skipped;
           raw out^T [d, i] and denominators (ones-matmul) accumulate on PE;
           both ship per head via an 8-rank AllToAll (head-split -> t-split).
  phase 3: normalize after the a2a, then y[t-slice, :] = outT^T @ wo in two
           per-collective passes.
Host assembles the 8 t-slices into the full (B, T, C) output.
"""

import os
import sys

import numpy as np

for _p in ("/opt/trn_rl_repo", "/root/.axon_site/_ro/trn_rl_repo"):
    if os.path.isdir(_p) and _p not in sys.path:
        sys.path.append(_p)

import ml_dtypes

import concourse.bacc as bacc
import concourse.tile as tile
from concourse import mybir
from concourse.bass_utils import run_bass_kernel_spmd

B, T, C = 2, 2048, 2048
N_HEADS, D = 16, 128
THETA = 10000.0
N_CORES = 8
HPC = N_HEADS // N_CORES     # heads per core
BT = B * T                   # flattened time axis
TSL = BT // N_CORES          # per-core output slice after the all-to-all
KT = C // 128                # contraction chunks
TC1 = 512                    # phase-1 t-chunk (moving free dim)
NTC1 = BT // TC1
TC2 = 512                    # phase-2/3 chunk
CI = T // TC2                # i-chunks per (head, batch)
JT = T // 128                # j-tiles per (head, batch)
SCALE = 1.0 / np.sqrt(D)
MASKED_BIAS = -1.0e6         # pre-scale units; exp(SCALE*(s+bias)) == 0

BF16 = mybir.dt.bfloat16
F32 = mybir.dt.float32
NPBF = ml_dtypes.bfloat16


def _bf16(a):
    return np.ascontiguousarray(np.asarray(a, dtype=np.float32)).astype(NPBF)


def _mask_plan(mask2d):
    """Per (ci, jt) code: None=skip (all masked), -1=free (none masked),
    >=0 = index of partial-mask bias tile. scoresT tile (jt, ci) holds
    mask2d[i, j] transposed: bias[j_loc, i_loc] <- mask2d[TC2*ci+i, 128*jt+j].
    """
    uniq = {}
    tiles = []
    plan = []
    for ci in range(CI):
        row = []
        for jt in range(JT):
            blk = mask2d[TC2 * ci:TC2 * (ci + 1), 128 * jt:128 * (jt + 1)]
            if blk.all():
                row.append(-1)
            elif not blk.any():
                row.append(None)
            else:
                bias = np.where(blk.T, 0.0, np.float32(MASKED_BIAS)).astype(np.float32)
                key = bias.tobytes()
                if key not in uniq:
                    uniq[key] = len(tiles)
                    tiles.append(bias)
                row.append(uniq[key])
        plan.append(row)
    if not tiles:  # keep the DRAM tensor non-empty
        tiles.append(np.zeros((128, TC2), np.float32))
    return plan, np.stack(tiles)


def _rope_tables():
    inv_freq = 1.0 / (THETA ** (np.arange(0, D, 2, dtype=np.float64) / D))
    freqs = np.outer(inv_freq, np.arange(T, dtype=np.float64))  # [64, T]
    cosI = np.repeat(np.cos(freqs), 2, axis=0).astype(np.float32)  # [128, T]
    sinI = np.repeat(np.sin(freqs), 2, axis=0).astype(np.float32)
    # rot = psignT.T @ x : rot[2i] = -x[2i+1], rot[2i+1] = x[2i]
    psignT = np.zeros((D, D), np.float32)
    for i in range(D // 2):
        psignT[2 * i + 1, 2 * i] = -1.0
        psignT[2 * i, 2 * i + 1] = 1.0
    return cosI, sinI, psignT


def _phase1(nc, tc, qkv_tensors, xT_r, cos_sb, sin_sb):
    qT, kT, vt, wq_h, wk_h, wv_sb, psg_sb = qkv_tensors  # w/cos/sin in wpool
    with tc.tile_pool(name="xt", bufs=2) as xp, \
         tc.tile_pool(name="p1t", bufs=1) as p1, \
         tc.tile_pool(name="ps1", bufs=1, space="PSUM") as pp:
        for tcn in range(NTC1):
            ts = tcn * TC1           # position in flattened BT
            tp = ts % T              # rope position (restarts per batch)
            xt = xp.tile([128, KT, TC1], BF16, tag="xt")
            # chunk 0 feeds the very first matmuls: split it finely across
            # TWO descriptor queues (sync + scalar) so its latency halves
            nparts = 8 if tcn == 0 else 2
            step = KT // nparts
            for q_ in range(nparts):
                eng = nc.scalar if (tcn == 0 and q_ % 2 == 1) else nc.sync
                eng.dma_start(xt[:, q_ * step:(q_ + 1) * step, :],
                              xT_r[:, q_ * step:(q_ + 1) * step,
                                   ts:ts + TC1])
            for dst, w_h in ((qT, wq_h), (kT, wk_h)):
                for h in range(HPC):
                    ps = pp.tile([D, TC1], F32, tag="proj", bufs=4)
                    for cc in range(KT):
                        nc.tensor.matmul(
                            ps[:], w_h[h][:, cc, :], xt[:, cc, :],
                            start=(cc == 0), stop=(cc == KT - 1))
                    praw = p1.tile([D, TC1], BF16, tag="praw", bufs=3)
                    nc.vector.tensor_copy(praw[:], ps[:])
                    rot = pp.tile([D, TC1], F32, tag="rot", bufs=2)
                    nc.tensor.matmul(rot[:], psg_sb[:], praw[:],
                                     start=True, stop=True)
                    t1 = p1.tile([D, TC1], BF16, tag="t1", bufs=2)
                    nc.vector.tensor_mul(t1[:], praw[:], cos_sb[:, tp:tp + TC1])
                    t2 = p1.tile([D, TC1], BF16, tag="t2", bufs=2)
                    nc.vector.tensor_mul(t2[:], rot[:], sin_sb[:, tp:tp + TC1])
                    nc.vector.tensor_add(dst[h][:, ts:ts + TC1], t1[:], t2[:])
            # v projection: out [t, d] per 128-row t-tile
            for tt in range(TC1 // 128):
                jt = ts // 128 + tt
                ps = pp.tile([128, HPC * D], F32, tag="vproj", bufs=2)
                for cc in range(KT):
                    nc.tensor.matmul(
                        ps[:], xt[:, cc, tt * 128:(tt + 1) * 128],
                        wv_sb[:, cc, :],
                        start=(cc == 0), stop=(cc == KT - 1))
                nc.vector.tensor_copy(vt[jt][:], ps[:])


def _attn_out(nc, tc, plan, bias_sb, qT, kT, vt, ones_sb,
              a2a_in, a2a_out, wop, wo_sb, y):
    """Phase 2 (attention per head + a2a) and phase 3 (normalize + wo),
    emission-interleaved so the second collective hides under pass-A
    compute: each collective's receive-side loads go out right after its
    trigger (ahead of later sync-queue work), and the k=0 normalize chain
    is emitted mid-head-1 so scaled a2a0 data is ready the moment the PE
    drains head 1."""
    with tc.tile_pool(name="p3", bufs=1) as aop:
        ao = {}      # (k, s) -> raw/scaled output tile
        recb = {}    # k -> bf16 reciprocal denominators [8, TC2]
        yA = [aop.tile([128, TC2], F32, name=f"yA{i}") for i in range(16)]

        def emit_recv(k):
            # receive-side DMAs for collective k, right behind its trigger
            for s in range(N_CORES):
                t_ = aop.tile([128, TC2], BF16, name=f"ao{k}_{s}")
                nc.sync.dma_start(t_[:], a2a_out[k][s, 0:D, :])
                ao[(k, s)] = t_
            den = aop.tile([N_CORES, TC2], BF16, name=f"den{k}")
            for s in range(N_CORES):
                nc.sync.dma_start(den[s:s + 1, :], a2a_out[k][s, D:D + 1, :])
            ao[(k, 'den')] = den

        def emit_norm(k):
            # reciprocal of the softmax denominators + in-place scaling
            den = ao[(k, 'den')]
            denf = aop.tile([N_CORES, TC2], F32, name=f"denf{k}")
            nc.vector.tensor_copy(denf[:], den[:])
            rec = aop.tile([N_CORES, TC2], F32, name=f"rec{k}")
            nc.vector.reciprocal(rec[:], denf[:])
            rb16 = aop.tile([N_CORES, TC2], BF16, name=f"recb{k}")
            nc.vector.tensor_copy(rb16[:], rec[:])
            recb[k] = rb16
            for s in range(N_CORES):
                # partition_broadcast needs its source at partition 0
                rec1 = aop.tile([1, TC2], BF16, tag="rstage", bufs=2)
                nc.sync.dma_start(rec1[:], rb16[s:s + 1, :])
                rb = aop.tile([128, TC2], BF16, tag="rb", bufs=2)
                nc.gpsimd.partition_broadcast(rb[:], rec1[:])
                nc.vector.tensor_mul(ao[(k, s)][:], ao[(k, s)][:], rb[:])

        _phase2(nc, tc, plan, bias_sb, qT, kT, vt, ones_sb, a2a_in, a2a_out,
                emit_recv, emit_norm)

        with tc.tile_pool(name="ps3", bufs=1, space="PSUM") as pp:
            emit_norm(1)
            for k in range(HPC):
                for cj in range(C // TC2):
                    for tt in range(TSL // 128):
                        idx = cj * (TSL // 128) + tt
                        yp = pp.tile([128, TC2], F32, tag="y", bufs=4)
                        for s in range(N_CORES):
                            nc.tensor.matmul(
                                yp[:], ao[(k, s)][:, tt * 128:(tt + 1) * 128],
                                wo_sb[:, s * HPC + k, cj, :],
                                start=(s == 0), stop=(s == N_CORES - 1))
                        if k == 0:
                            # scalar engine is idle here; keep vector free
                            # for the k=1 normalize chain
                            nc.scalar.copy(yA[idx][:], yp[:])
                        else:
                            ysb = wop.tile([128, TC2], F32, tag="ysb", bufs=3)
                            nc.vector.tensor_add(ysb[:], yp[:], yA[idx][:])
                            nc.sync.dma_start(
                                y[tt * 128:(tt + 1) * 128,
                                  cj * TC2:(cj + 1) * TC2],
                                ysb[:])


def _phase2(nc, tc, plan, bias_sb, qT, kT, vt, ones_sb,
            a2a_in, a2a_out, emit_recv, emit_norm):
    with tc.tile_pool(name="p2t", bufs=1) as p2, \
         tc.tile_pool(name="ps2", bufs=1, space="PSUM") as pp:
        for h in range(HPC):
            for b in range(B):
                if h == 1 and b == 1:
                    # a2a0 has landed by now: normalize its payload while
                    # the PE is still busy with head 1 so pass A can start
                    # the instant head 1 drains (hiding a2a1)
                    emit_norm(0)
                for ci in range(CI):
                    gci = b * CI + ci      # global chunk == dest rank
                    live = [(jt, plan[ci][jt]) for jt in range(JT)
                            if plan[ci][jt] is not None]
                    if not live:
                        z = p2.tile([128, TC2], BF16, tag="ot", bufs=3)
                        nc.vector.memset(z[:], 0.0)
                        nc.sync.dma_start(a2a_in[h][gci, 0:D, :], z[:])
                        zd = p2.tile([1, TC2], BF16, tag="rps", bufs=2)
                        nc.vector.memset(zd[:], 1.0)
                        nc.sync.dma_start(a2a_in[h][gci, D:D + 1, :], zd[:])
                        continue
                    outp = pp.tile([D, TC2], F32, tag="outT", bufs=3)
                    rp = pp.tile([1, TC2], F32, tag="r", bufs=2)
                    i0 = b * T + ci * TC2
                    qs = qT[h][:, i0:i0 + TC2]
                    for idx, (jt, code) in enumerate(live):
                        jv = (b * T) // 128 + jt
                        sc = pp.tile([128, TC2], F32, tag="sc", bufs=3)
                        nc.tensor.matmul(
                            sc[:],
                            kT[h][:, b * T + jt * 128:b * T + (jt + 1) * 128],
                            qs, start=True, stop=True)
                        if code >= 0:
                            mt = p2.tile([128, TC2], F32, tag="mt", bufs=2)
                            nc.vector.tensor_add(mt[:], sc[:],
                                                 bias_sb[:, code, :])
                            src = mt
                        else:
                            src = sc
                        pt = p2.tile([128, TC2], BF16, tag="pt", bufs=4)
                        nc.scalar.activation(
                            pt[:], src[:], mybir.ActivationFunctionType.Exp,
                            bias=0.0, scale=float(SCALE))
                        nc.tensor.matmul(
                            outp[:], vt[jv][:, h * D:(h + 1) * D], pt[:],
                            start=(idx == 0), stop=(idx == len(live) - 1))
                        nc.tensor.matmul(
                            rp[:], ones_sb[:], pt[:],
                            start=(idx == 0), stop=(idx == len(live) - 1))
                    # ship raw output + denominator; normalize after the a2a
                    ot = p2.tile([128, TC2], BF16, tag="ot", bufs=3)
                    nc.vector.tensor_copy(ot[:], outp[:])
                    nc.sync.dma_start(a2a_in[h][gci, 0:D, :], ot[:])
                    rps = p2.tile([1, TC2], BF16, tag="rps", bufs=2)
                    nc.vector.tensor_copy(rps[:], rp[:])
                    nc.sync.dma_start(a2a_in[h][gci, D:D + 1, :], rps[:])
            # this head's comm overlaps the next head's compute
            nc.gpsimd.collective_compute(
                "AllToAll", mybir.AluOpType.bypass,
                replica_groups=[list(range(N_CORES))],
                ins=[a2a_in[h].opt()], outs=[a2a_out[h].opt()])
            # queue the receive-side loads NOW so they sit ahead of the
            # next head's output DMAs on the sync queue
            emit_recv(h)


def _build(plan, n_bias):
    nc = bacc.Bacc("TRN2", num_devices=N_CORES)

    xT = nc.dram_tensor("xT", [C, BT], BF16, kind="ExternalInput")
    wq = nc.dram_tensor("wq", [C, HPC * D], BF16, kind="ExternalInput")
    wk = nc.dram_tensor("wk", [C, HPC * D], BF16, kind="ExternalInput")
    wv = nc.dram_tensor("wv", [C, HPC * D], BF16, kind="ExternalInput")
    wo = nc.dram_tensor("wo", [N_HEADS * D, C], BF16, kind="ExternalInput")
    cos_d = nc.dram_tensor("cos", [D, T], BF16, kind="ExternalInput")
    sin_d = nc.dram_tensor("sin", [D, T], BF16, kind="ExternalInput")
    psg_d = nc.dram_tensor("psg", [D, D], BF16, kind="ExternalInput")
    ones_d = nc.dram_tensor("ones", [128, 1], BF16, kind="ExternalInput")
    bias_d = nc.dram_tensor("bias", [n_bias, 128, TC2], F32, kind="ExternalInput")
    y = nc.dram_tensor("y", [TSL, C], F32, kind="ExternalOutput")

    xT_r = xT.rearrange("(n p) t -> p n t", p=128)
    wq_r = wq.rearrange("(n p) (h d) -> p n h d", p=128, d=D)
    wk_r = wk.rearrange("(n p) (h d) -> p n h d", p=128, d=D)
    wo_r = wo.rearrange("(n p) (cb m) -> p n cb m", p=128, m=TC2)

    with tile.TileContext(nc) as tc:
        with tc.tile_pool(name="const", bufs=1) as cpool, \
             tc.tile_pool(name="dram", bufs=1, space="DRAM") as dram:

            a2a_in = [dram.tile([N_CORES, D + 1, TC2], BF16, name=f"a2ai{h}")
                      for h in range(HPC)]
            a2a_out = [dram.tile([N_CORES, D + 1, TC2], BF16, name=f"a2ao{h}")
                       for h in range(HPC)]

            with tc.tile_pool(name="qkv", bufs=1) as qkv:
                qT = [qkv.tile([D, BT], BF16, name=f"qT{h}") for h in range(HPC)]
                kT = [qkv.tile([D, BT], BF16, name=f"kT{h}") for h in range(HPC)]
                vt = [qkv.tile([128, HPC * D], BF16, name=f"v{j}")
                      for j in range(BT // 128)]

                with tc.tile_pool(name="wp", bufs=1) as wp:
                    # startup: spread descriptor generation across engine
                    # queues -- sync only carries the xt chunks so the first
                    # matmul's data is in flight immediately.
                    wq_h = []
                    for h in range(HPC):
                        w_ = wp.tile([128, KT, D], BF16, name=f"wqh{h}")
                        for q_ in range(4):
                            nc.scalar.dma_start(
                                w_[:, 4 * q_:4 * (q_ + 1), :],
                                wq_r[:, 4 * q_:4 * (q_ + 1), h, :])
                        wq_h.append(w_)
                    psg_sb = cpool.tile([D, D], BF16)
                    nc.gpsimd.dma_start(psg_sb[:], psg_d[:])
                    ones_sb = cpool.tile([128, 1], BF16)
                    nc.gpsimd.dma_start(ones_sb[:], ones_d[:])
                    warm = cpool.tile([128, 1], F32)
                    nc.scalar.activation(warm[:], ones_sb[:],
                                         mybir.ActivationFunctionType.Exp,
                                         bias=0.0, scale=1.0)
                    warm2 = cpool.tile([128, 1], BF16)
                    nc.gpsimd.partition_broadcast(warm2[:], ones_sb[0:1, :])
                    wk_h = []
                    for h in range(HPC):
                        w_ = wp.tile([128, KT, D], BF16, name=f"wkh{h}")
                        nc.gpsimd.dma_start(w_[:, 0:KT // 2, :],
                                            wk_r[:, 0:KT // 2, h, :])
                        nc.gpsimd.dma_start(w_[:, KT // 2:KT, :],
                                            wk_r[:, KT // 2:KT, h, :])
                        wk_h.append(w_)
                    cos_sb = wp.tile([D, T], BF16)
                    nc.gpsimd.dma_start(cos_sb[:], cos_d[:])
                    sin_sb = wp.tile([D, T], BF16)
                    nc.gpsimd.dma_start(sin_sb[:], sin_d[:])
                    wv_sb = wp.tile([128, KT, HPC * D], BF16)
                    nc.scalar.dma_start(wv_sb[:],
                                        wv.rearrange("(n p) m -> p n m", p=128))

                    _phase1(nc, tc, (qT, kT, vt, wq_h, wk_h, wv_sb, psg_sb),
                            xT_r, cos_sb, sin_sb)

                # wo pool opens as soon as the phase-1 weights are freed; the
                # full wo (8 MB bf16) + bias prefetch on the idle gpsimd queue
                # so they land under phase-2 compute
                with tc.tile_pool(name="wo", bufs=1) as wop:
                    bias_sb = wop.tile([128, n_bias, TC2], F32)
                    nc.gpsimd.dma_start(bias_sb[:],
                                        bias_d.rearrange("u p m -> p u m"))
                    wo_sb = wop.tile([128, KT, C // TC2, TC2], BF16)
                    for q_ in range(8):
                        nc.gpsimd.dma_start(
                            wo_sb[:, 2 * q_:2 * (q_ + 1), :, :],
                            wo_r[:, 2 * q_:2 * (q_ + 1), :, :])
                    _attn_out(nc, tc, plan, bias_sb, qT, kT, vt,
                              ones_sb, a2a_in, a2a_out, wop, wo_sb, y)

    nc.finalize()
    return nc


_cache = {}


def _get_kernel(mask2d):
    key = mask2d.tobytes()
    if key not in _cache:
        plan, bias_tiles = _mask_plan(mask2d)
        nc = _build(plan, bias_tiles.shape[0])
        _cache[key] = (nc, bias_tiles)
    return _cache[key]


def kernel(x, mask, wq, wk, wv, wo, _trace=False):
    x = np.asarray(x)
    mask2d = np.asarray(mask).reshape(T, T).astype(bool)
    nc, bias_tiles = _get_kernel(mask2d)

    cosI, sinI, psignT = _rope_tables()
    xT_full = _bf16(np.asarray(x).reshape(BT, C).T)
    common = {
        "cos": _bf16(cosI), "sin": _bf16(sinI), "psg": _bf16(psignT),
        "ones": np.ones((128, 1), NPBF),
        "bias": bias_tiles, "wo": _bf16(wo), "xT": xT_full,
    }
    in_maps = []
    for c in range(N_CORES):
        sl = slice(c * HPC * D, (c + 1) * HPC * D)
        in_maps.append({
            "wq": _bf16(np.asarray(wq)[:, sl]),
            "wk": _bf16(np.asarray(wk)[:, sl]),
            "wv": _bf16(np.asarray(wv)[:, sl]),
            **common,
        })

    r = run_bass_kernel_spmd(nc, in_maps, core_ids=list(range(N_CORES)),
                             trace=_trace)
    out = np.empty((BT, C), np.float32)
    for c in range(N_CORES):
        out[c * TSL:(c + 1) * TSL, :] = r.results[c]["y"]
    if _trace:
        kernel.last_results = r
    return out.reshape(B, T, C)


# revision 11
# speedup vs baseline: 1.0305x; 1.0305x over previous
"""Multi-head causal attention (RoPE) forward on 8 Trainium2 NeuronCores.

Sharding: tensor-parallel over heads -- 8 cores x 2 heads, each core handling
both batch elements (the flattened (B*T) = 4096 "time" axis).

v2: all SBUF-resident data and the all-to-all payload are bfloat16 (PE speed
is the same 1 col/cycle as float32r, but HBM traffic, collective bytes and
SBUF pressure halve; DVE gets 2x mode). Startup DMA descriptor generation is
spread across the scalar/vector/gpsimd queues so the first xt chunk is not
serialized behind ~10 MB of weight loads on the sync queue. Attention output
leaves the core UNnormalized with the softmax denominators as a 129th row of
the a2a payload; normalization happens after the a2a (one reciprocal per
source instead of 16, and the phase-2 critical path drops the
recip/broadcast/mul chain). Phase 3 runs in two passes -- the 8 matmul
contributions of a2a0 are accumulated and parked in SBUF while a2a1 is still
in flight, then pass B adds the rest; wo is prefetched in full (8 MB bf16)
during phase 2.

Per core:
  phase 1 (TC1=512 chunks): qT/kT [d, B*T] and v [B*T, d] projections from
           host-pre-transposed xT, RoPE via a +-1 pair-swap permutation
           matmul on PE plus elementwise combine with interleaved cos/sin.
  phase 2: per (head, batch), scores^T [j, i] = kT^T @ qT, exp on ScalarE
           (no max pass -- bounded score distribution), mask as additive
           bias on partially-masked tiles only, fully-masked tiles skipped;
           raw out^T [d, i] and denominators (ones-matmul) accumulate on PE;
           both ship per head via an 8-rank AllToAll (head-split -> t-split).
  phase 3: normalize after the a2a, then y[t-slice, :] = outT^T @ wo in two
           per-collective passes.
Host assembles the 8 t-slices into the full (B, T, C) output.
"""

import os
import sys

import numpy as np

for _p in ("/opt/trn_rl_repo", "/root/.axon_site/_ro/trn_rl_repo"):
    if os.path.isdir(_p) and _p not in sys.path:
        sys.path.append(_p)

import ml_dtypes

import concourse.bacc as bacc
import concourse.tile as tile
from concourse import mybir
from concourse.bass_utils import run_bass_kernel_spmd

B, T, C = 2, 2048, 2048
N_HEADS, D = 16, 128
THETA = 10000.0
N_CORES = 8
HPC = N_HEADS // N_CORES     # heads per core
BT = B * T                   # flattened time axis
TSL = BT // N_CORES          # per-core output slice after the all-to-all
KT = C // 128                # contraction chunks
TC1 = 512                    # phase-1 t-chunk (moving free dim)
NTC1 = BT // TC1
TC2 = 512                    # phase-2/3 chunk
CI = T // TC2                # i-chunks per (head, batch)
JT = T // 128                # j-tiles per (head, batch)
SCALE = 1.0 / np.sqrt(D)
MASKED_BIAS = -1.0e6         # pre-scale units; exp(SCALE*(s+bias)) == 0

BF16 = mybir.dt.bfloat16
F32 = mybir.dt.float32
NPBF = ml_dtypes.bfloat16


def _bf16(a):
    return np.ascontiguousarray(np.asarray(a, dtype=np.float32)).astype(NPBF)


def _mask_plan(mask2d):
    """Per (ci, jt) code: None=skip (all masked), -1=free (none masked),
    >=0 = index of partial-mask bias tile. scoresT tile (jt, ci) holds
    mask2d[i, j] transposed: bias[j_loc, i_loc] <- mask2d[TC2*ci+i, 128*jt+j].
    """
    uniq = {}
    tiles = []
    plan = []
    for ci in range(CI):
        row = []
        for jt in range(JT):
            blk = mask2d[TC2 * ci:TC2 * (ci + 1), 128 * jt:128 * (jt + 1)]
            if blk.all():
                row.append(-1)
            elif not blk.any():
                row.append(None)
            else:
                bias = np.where(blk.T, 0.0, np.float32(MASKED_BIAS)).astype(np.float32)
                key = bias.tobytes()
                if key not in uniq:
                    uniq[key] = len(tiles)
                    tiles.append(bias)
                row.append(uniq[key])
        plan.append(row)
    if not tiles:  # keep the DRAM tensor non-empty
        tiles.append(np.zeros((128, TC2), np.float32))
    return plan, np.stack(tiles)


def _rope_tables():
    inv_freq = 1.0 / (THETA ** (np.arange(0, D, 2, dtype=np.float64) / D))
    freqs = np.outer(inv_freq, np.arange(T, dtype=np.float64))  # [64, T]
    cosI = np.repeat(np.cos(freqs), 2, axis=0).astype(np.float32)  # [128, T]
    sinI = np.repeat(np.sin(freqs), 2, axis=0).astype(np.float32)
    # rot = psignT.T @ x : rot[2i] = -x[2i+1], rot[2i+1] = x[2i]
    psignT = np.zeros((D, D), np.float32)
    for i in range(D // 2):
        psignT[2 * i + 1, 2 * i] = -1.0
        psignT[2 * i, 2 * i + 1] = 1.0
    return cosI, sinI, psignT


def _phase1(nc, tc, qkv_tensors, xT_r, cos_sb, sin_sb):
    qT, kT, vt, wq_h, wk_h, wv_sb, psg_sb = qkv_tensors  # w/cos/sin in wpool
    with tc.tile_pool(name="xt", bufs=2) as xp, \
         tc.tile_pool(name="p1t", bufs=1) as p1, \
         tc.tile_pool(name="ps1", bufs=1, space="PSUM") as pp:
        for tcn in range(NTC1):
            ts = tcn * TC1           # position in flattened BT
            tp = ts % T              # rope position (restarts per batch)
            xt = xp.tile([128, KT, TC1], BF16, tag="xt")
            nparts = 4 if tcn == 0 else 2
            step = KT // nparts
            for q_ in range(nparts):
                nc.sync.dma_start(xt[:, q_ * step:(q_ + 1) * step, :],
                                  xT_r[:, q_ * step:(q_ + 1) * step,
                                       ts:ts + TC1])
            for dst, w_h in ((qT, wq_h), (kT, wk_h)):
                for h in range(HPC):
                    ps = pp.tile([D, TC1], F32, tag="proj", bufs=4)
                    for cc in range(KT):
                        nc.tensor.matmul(
                            ps[:], w_h[h][:, cc, :], xt[:, cc, :],
                            start=(cc == 0), stop=(cc == KT - 1))
                    praw = p1.tile([D, TC1], BF16, tag="praw", bufs=3)
                    nc.vector.tensor_copy(praw[:], ps[:])
                    rot = pp.tile([D, TC1], F32, tag="rot", bufs=2)
                    nc.tensor.matmul(rot[:], psg_sb[:], praw[:],
                                     start=True, stop=True)
                    t1 = p1.tile([D, TC1], BF16, tag="t1", bufs=2)
                    nc.vector.tensor_mul(t1[:], praw[:], cos_sb[:, tp:tp + TC1])
                    t2 = p1.tile([D, TC1], BF16, tag="t2", bufs=2)
                    nc.vector.tensor_mul(t2[:], rot[:], sin_sb[:, tp:tp + TC1])
                    nc.vector.tensor_add(dst[h][:, ts:ts + TC1], t1[:], t2[:])
            # v projection: out [t, d] per 128-row t-tile
            for tt in range(TC1 // 128):
                jt = ts // 128 + tt
                ps = pp.tile([128, HPC * D], F32, tag="vproj", bufs=2)
                for cc in range(KT):
                    nc.tensor.matmul(
                        ps[:], xt[:, cc, tt * 128:(tt + 1) * 128],
                        wv_sb[:, cc, :],
                        start=(cc == 0), stop=(cc == KT - 1))
                nc.vector.tensor_copy(vt[jt][:], ps[:])


def _attn_out(nc, tc, plan, bias_sb, qT, kT, vt, ones_sb,
              a2a_in, a2a_out, wop, wo_sb, y):
    """Phase 2 (attention per head + a2a) and phase 3 (normalize + wo),
    emission-interleaved so the second collective hides under pass-A
    compute: each collective's receive-side loads go out right after its
    trigger (ahead of later sync-queue work), and the k=0 normalize chain
    is emitted mid-head-1 so scaled a2a0 data is ready the moment the PE
    drains head 1."""
    with tc.tile_pool(name="p3", bufs=1) as aop:
        ao = {}      # (k, s) -> raw/scaled output tile
        recb = {}    # k -> bf16 reciprocal denominators [8, TC2]
        yA = [aop.tile([128, TC2], F32, name=f"yA{i}") for i in range(16)]

        def emit_recv(k):
            # receive-side DMAs for collective k, right behind its trigger
            for s in range(N_CORES):
                t_ = aop.tile([128, TC2], BF16, name=f"ao{k}_{s}")
                nc.sync.dma_start(t_[:], a2a_out[k][s, 0:D, :])
                ao[(k, s)] = t_
            den = aop.tile([N_CORES, TC2], BF16, name=f"den{k}")
            for s in range(N_CORES):
                nc.sync.dma_start(den[s:s + 1, :], a2a_out[k][s, D:D + 1, :])
            ao[(k, 'den')] = den

        def emit_norm(k):
            # reciprocal of the softmax denominators + in-place scaling
            den = ao[(k, 'den')]
            denf = aop.tile([N_CORES, TC2], F32, name=f"denf{k}")
            nc.vector.tensor_copy(denf[:], den[:])
            rec = aop.tile([N_CORES, TC2], F32, name=f"rec{k}")
            nc.vector.reciprocal(rec[:], denf[:])
            rb16 = aop.tile([N_CORES, TC2], BF16, name=f"recb{k}")
            nc.vector.tensor_copy(rb16[:], rec[:])
            recb[k] = rb16
            for s in range(N_CORES):
                # partition_broadcast needs its source at partition 0
                rec1 = aop.tile([1, TC2], BF16, tag="rstage", bufs=2)
                nc.sync.dma_start(rec1[:], rb16[s:s + 1, :])
                rb = aop.tile([128, TC2], BF16, tag="rb", bufs=2)
                nc.gpsimd.partition_broadcast(rb[:], rec1[:])
                nc.vector.tensor_mul(ao[(k, s)][:], ao[(k, s)][:], rb[:])

        _phase2(nc, tc, plan, bias_sb, qT, kT, vt, ones_sb, a2a_in, a2a_out,
                emit_recv, emit_norm)

        with tc.tile_pool(name="ps3", bufs=1, space="PSUM") as pp:
            emit_norm(1)
            for k in range(HPC):
                for cj in range(C // TC2):
                    for tt in range(TSL // 128):
                        idx = cj * (TSL // 128) + tt
                        yp = pp.tile([128, TC2], F32, tag="y", bufs=4)
                        for s in range(N_CORES):
                            nc.tensor.matmul(
                                yp[:], ao[(k, s)][:, tt * 128:(tt + 1) * 128],
                                wo_sb[:, s * HPC + k, cj, :],
                                start=(s == 0), stop=(s == N_CORES - 1))
                        if k == 0:
                            # scalar engine is idle here; keep vector free
                            # for the k=1 normalize chain
                            nc.scalar.copy(yA[idx][:], yp[:])
                        else:
                            ysb = wop.tile([128, TC2], F32, tag="ysb", bufs=3)
                            nc.vector.tensor_add(ysb[:], yp[:], yA[idx][:])
                            nc.sync.dma_start(
                                y[tt * 128:(tt + 1) * 128,
                                  cj * TC2:(cj + 1) * TC2],
                                ysb[:])


def _phase2(nc, tc, plan, bias_sb, qT, kT, vt, ones_sb,
            a2a_in, a2a_out, emit_recv, emit_norm):
    with tc.tile_pool(name="p2t", bufs=1) as p2, \
         tc.tile_pool(name="ps2", bufs=1, space="PSUM") as pp:
        for h in range(HPC):
            for b in range(B):
                for ci in range(CI):
                    if h == 1 and b == 1 and ci == 2:
                        # a2a0 has landed by now: normalize its payload
                        # while the PE is still busy with head 1 so pass A
                        # can start the instant head 1 drains (hiding a2a1)
                        emit_norm(0)
                    gci = b * CI + ci      # global chunk == dest rank
                    live = [(jt, plan[ci][jt]) for jt in range(JT)
                            if plan[ci][jt] is not None]
                    if not live:
                        z = p2.tile([128, TC2], BF16, tag="ot", bufs=3)
                        nc.vector.memset(z[:], 0.0)
                        nc.sync.dma_start(a2a_in[h][gci, 0:D, :], z[:])
                        zd = p2.tile([1, TC2], BF16, tag="rps", bufs=2)
                        nc.vector.memset(zd[:], 1.0)
                        nc.sync.dma_start(a2a_in[h][gci, D:D + 1, :], zd[:])
                        continue
                    outp = pp.tile([D, TC2], F32, tag="outT", bufs=3)
                    rp = pp.tile([1, TC2], F32, tag="r", bufs=2)
                    i0 = b * T + ci * TC2
                    qs = qT[h][:, i0:i0 + TC2]
                    for idx, (jt, code) in enumerate(live):
                        jv = (b * T) // 128 + jt
                        sc = pp.tile([128, TC2], F32, tag="sc", bufs=3)
                        nc.tensor.matmul(
                            sc[:],
                            kT[h][:, b * T + jt * 128:b * T + (jt + 1) * 128],
                            qs, start=True, stop=True)
                        if code >= 0:
                            mt = p2.tile([128, TC2], F32, tag="mt", bufs=2)
                            nc.vector.tensor_add(mt[:], sc[:],
                                                 bias_sb[:, code, :])
                            src = mt
                        else:
                            src = sc
                        pt = p2.tile([128, TC2], BF16, tag="pt", bufs=4)
                        nc.scalar.activation(
                            pt[:], src[:], mybir.ActivationFunctionType.Exp,
                            bias=0.0, scale=float(SCALE))
                        nc.tensor.matmul(
                            outp[:], vt[jv][:, h * D:(h + 1) * D], pt[:],
                            start=(idx == 0), stop=(idx == len(live) - 1))
                        nc.tensor.matmul(
                            rp[:], ones_sb[:], pt[:],
                            start=(idx == 0), stop=(idx == len(live) - 1))
                    # ship raw output + denominator; normalize after the a2a
                    ot = p2.tile([128, TC2], BF16, tag="ot", bufs=3)
                    nc.vector.tensor_copy(ot[:], outp[:])
                    nc.sync.dma_start(a2a_in[h][gci, 0:D, :], ot[:])
                    rps = p2.tile([1, TC2], BF16, tag="rps", bufs=2)
                    nc.vector.tensor_copy(rps[:], rp[:])
                    nc.sync.dma_start(a2a_in[h][gci, D:D + 1, :], rps[:])
            # this head's comm overlaps the next head's compute
            nc.gpsimd.collective_compute(
                "AllToAll", mybir.AluOpType.bypass,
                replica_groups=[list(range(N_CORES))],
                ins=[a2a_in[h].opt()], outs=[a2a_out[h].opt()])
            # queue the receive-side loads NOW so they sit ahead of the
            # next head's output DMAs on the sync queue
            emit_recv(h)


def _build(plan, n_bias):
    nc = bacc.Bacc("TRN2", num_devices=N_CORES)

    xT = nc.dram_tensor("xT", [C, BT], BF16, kind="ExternalInput")
    wq = nc.dram_tensor("wq", [C, HPC * D], BF16, kind="ExternalInput")
    wk = nc.dram_tensor("wk", [C, HPC * D], BF16, kind="ExternalInput")
    wv = nc.dram_tensor("wv", [C, HPC * D], BF16, kind="ExternalInput")
    wo = nc.dram_tensor("wo", [N_HEADS * D, C], BF16, kind="ExternalInput")
    cos_d = nc.dram_tensor("cos", [D, T], BF16, kind="ExternalInput")
    sin_d = nc.dram_tensor("sin", [D, T], BF16, kind="ExternalInput")
    psg_d = nc.dram_tensor("psg", [D, D], BF16, kind="ExternalInput")
    ones_d = nc.dram_tensor("ones", [128, 1], BF16, kind="ExternalInput")
    bias_d = nc.dram_tensor("bias", [n_bias, 128, TC2], F32, kind="ExternalInput")
    y = nc.dram_tensor("y", [TSL, C], F32, kind="ExternalOutput")

    xT_r = xT.rearrange("(n p) t -> p n t", p=128)
    wq_r = wq.rearrange("(n p) (h d) -> p n h d", p=128, d=D)
    wk_r = wk.rearrange("(n p) (h d) -> p n h d", p=128, d=D)
    wo_r = wo.rearrange("(n p) (cb m) -> p n cb m", p=128, m=TC2)

    with tile.TileContext(nc) as tc:
        with tc.tile_pool(name="const", bufs=1) as cpool, \
             tc.tile_pool(name="dram", bufs=1, space="DRAM") as dram:

            a2a_in = [dram.tile([N_CORES, D + 1, TC2], BF16, name=f"a2ai{h}")
                      for h in range(HPC)]
            a2a_out = [dram.tile([N_CORES, D + 1, TC2], BF16, name=f"a2ao{h}")
                       for h in range(HPC)]

            with tc.tile_pool(name="qkv", bufs=1) as qkv:
                qT = [qkv.tile([D, BT], BF16, name=f"qT{h}") for h in range(HPC)]
                kT = [qkv.tile([D, BT], BF16, name=f"kT{h}") for h in range(HPC)]
                vt = [qkv.tile([128, HPC * D], BF16, name=f"v{j}")
                      for j in range(BT // 128)]

                with tc.tile_pool(name="wp", bufs=1) as wp:
                    # startup: spread descriptor generation across engine
                    # queues -- sync only carries the xt chunks so the first
                    # matmul's data is in flight immediately.
                    wq_h = []
                    for h in range(HPC):
                        w_ = wp.tile([128, KT, D], BF16, name=f"wqh{h}")
                        for q_ in range(4):
                            nc.scalar.dma_start(
                                w_[:, 4 * q_:4 * (q_ + 1), :],
                                wq_r[:, 4 * q_:4 * (q_ + 1), h, :])
                        wq_h.append(w_)
                    psg_sb = cpool.tile([D, D], BF16)
                    nc.gpsimd.dma_start(psg_sb[:], psg_d[:])
                    ones_sb = cpool.tile([128, 1], BF16)
                    nc.gpsimd.dma_start(ones_sb[:], ones_d[:])
                    warm = cpool.tile([128, 1], F32)
                    nc.scalar.activation(warm[:], ones_sb[:],
                                         mybir.ActivationFunctionType.Exp,
                                         bias=0.0, scale=1.0)
                    warm2 = cpool.tile([128, 1], BF16)
                    nc.gpsimd.partition_broadcast(warm2[:], ones_sb[0:1, :])
                    wk_h = []
                    for h in range(HPC):
                        w_ = wp.tile([128, KT, D], BF16, name=f"wkh{h}")
                        nc.gpsimd.dma_start(w_[:, 0:KT // 2, :],
                                            wk_r[:, 0:KT // 2, h, :])
                        nc.gpsimd.dma_start(w_[:, KT // 2:KT, :],
                                            wk_r[:, KT // 2:KT, h, :])
                        wk_h.append(w_)
                    cos_sb = wp.tile([D, T], BF16)
                    nc.gpsimd.dma_start(cos_sb[:], cos_d[:])
                    sin_sb = wp.tile([D, T], BF16)
                    nc.gpsimd.dma_start(sin_sb[:], sin_d[:])
                    wv_sb = wp.tile([128, KT, HPC * D], BF16)
                    nc.scalar.dma_start(wv_sb[:],
                                        wv.rearrange("(n p) m -> p n m", p=128))

                    _phase1(nc, tc, (qT, kT, vt, wq_h, wk_h, wv_sb, psg_sb),
                            xT_r, cos_sb, sin_sb)

                # wo pool opens as soon as the phase-1 weights are freed; the
                # full wo (8 MB bf16) + bias prefetch on the idle gpsimd queue
                # so they land under phase-2 compute
                with tc.tile_pool(name="wo", bufs=1) as wop:
                    bias_sb = wop.tile([128, n_bias, TC2], F32)
                    nc.gpsimd.dma_start(bias_sb[:],
                                        bias_d.rearrange("u p m -> p u m"))
                    wo_sb = wop.tile([128, KT, C // TC2, TC2], BF16)
                    for q_ in range(8):
                        nc.gpsimd.dma_start(
                            wo_sb[:, 2 * q_:2 * (q_ + 1), :, :],
                            wo_r[:, 2 * q_:2 * (q_ + 1), :, :])
                    _attn_out(nc, tc, plan, bias_sb, qT, kT, vt,
                              ones_sb, a2a_in, a2a_out, wop, wo_sb, y)

    nc.finalize()
    return nc


_cache = {}


def _get_kernel(mask2d):
    key = mask2d.tobytes()
    if key not in _cache:
        plan, bias_tiles = _mask_plan(mask2d)
        nc = _build(plan, bias_tiles.shape[0])
        _cache[key] = (nc, bias_tiles)
    return _cache[key]


def kernel(x, mask, wq, wk, wv, wo, _trace=False):
    x = np.asarray(x)
    mask2d = np.asarray(mask).reshape(T, T).astype(bool)
    nc, bias_tiles = _get_kernel(mask2d)

    cosI, sinI, psignT = _rope_tables()
    xT_full = _bf16(np.asarray(x).reshape(BT, C).T)
    common = {
        "cos": _bf16(cosI), "sin": _bf16(sinI), "psg": _bf16(psignT),
        "ones": np.ones((128, 1), NPBF),
        "bias": bias_tiles, "wo": _bf16(wo), "xT": xT_full,
    }
    in_maps = []
    for c in range(N_CORES):
        sl = slice(c * HPC * D, (c + 1) * HPC * D)
        in_maps.append({
            "wq": _bf16(np.asarray(wq)[:, sl]),
            "wk": _bf16(np.asarray(wk)[:, sl]),
            "wv": _bf16(np.asarray(wv)[:, sl]),
            **common,
        })

    r = run_bass_kernel_spmd(nc, in_maps, core_ids=list(range(N_CORES)),
                             trace=_trace)
    out = np.empty((BT, C), np.float32)
    for c in range(N_CORES):
        out[c * TSL:(c + 1) * TSL, :] = r.results[c]["y"]
    if _trace:
        kernel.last_results = r
    return out.reshape(B, T, C)


# revision 15
# speedup vs baseline: 1.0441x; 1.0132x over previous
"""Multi-head causal attention (RoPE) forward on 8 Trainium2 NeuronCores.

Sharding: tensor-parallel over heads -- 8 cores x 2 heads, each core handling
both batch elements (the flattened (B*T) = 4096 "time" axis).

v2: all SBUF-resident data and the all-to-all payload are bfloat16 (PE speed
is the same 1 col/cycle as float32r, but HBM traffic, collective bytes and
SBUF pressure halve; DVE gets 2x mode). Startup DMA descriptor generation is
spread across the scalar/vector/gpsimd queues so the first xt chunk is not
serialized behind ~10 MB of weight loads on the sync queue. Attention output
leaves the core UNnormalized with the softmax denominators as a 129th row of
the a2a payload; normalization happens after the a2a (one reciprocal per
source instead of 16, and the phase-2 critical path drops the
recip/broadcast/mul chain). Phase 3 runs in two passes -- the 8 matmul
contributions of a2a0 are accumulated and parked in SBUF while a2a1 is still
in flight, then pass B adds the rest; wo is prefetched in full (8 MB bf16)
during phase 2.

Per core:
  phase 1 (TC1=512 chunks): qT/kT [d, B*T] and v [B*T, d] projections from
           host-pre-transposed xT, RoPE via a +-1 pair-swap permutation
           matmul on PE plus elementwise combine with interleaved cos/sin.
  phase 2: per (head, batch), scores^T [j, i] = kT^T @ qT, exp on ScalarE
           (no max pass -- bounded score distribution), mask as additive
           bias on partially-masked tiles only, fully-masked tiles skipped;
           raw out^T [d, i] and denominators (ones-matmul) accumulate on PE;
           both ship per head via an 8-rank AllToAll (head-split -> t-split).
  phase 3: normalize after the a2a, then y[t-slice, :] = outT^T @ wo in two
           per-collective passes.
Host assembles the 8 t-slices into the full (B, T, C) output.
"""

import os
import sys

import numpy as np

for _p in ("/opt/trn_rl_repo", "/root/.axon_site/_ro/trn_rl_repo"):
    if os.path.isdir(_p) and _p not in sys.path:
        sys.path.append(_p)

import ml_dtypes

import concourse.bacc as bacc
import concourse.tile as tile
from concourse import mybir
from concourse.bass_utils import run_bass_kernel_spmd

B, T, C = 2, 2048, 2048
N_HEADS, D = 16, 128
THETA = 10000.0
N_CORES = 8
HPC = N_HEADS // N_CORES     # heads per core
BT = B * T                   # flattened time axis
TSL = BT // N_CORES          # per-core output slice after the all-to-all
KT = C // 128                # contraction chunks
TC1 = 512                    # phase-1 t-chunk (moving free dim)
NTC1 = BT // TC1
TC2 = 512                    # phase-2/3 chunk
CI = T // TC2                # i-chunks per (head, batch)
JT = T // 128                # j-tiles per (head, batch)
SCALE = 1.0 / np.sqrt(D)
MASKED_BIAS = -1.0e6         # pre-scale units; exp(SCALE*(s+bias)) == 0

BF16 = mybir.dt.bfloat16
F32 = mybir.dt.float32
NPBF = ml_dtypes.bfloat16


def _bf16(a):
    return np.ascontiguousarray(np.asarray(a, dtype=np.float32)).astype(NPBF)


def _mask_plan(mask2d):
    """Per (ci, jt) code: None=skip (all masked), -1=free (none masked),
    >=0 = index of partial-mask bias tile. scoresT tile (jt, ci) holds
    mask2d[i, j] transposed: bias[j_loc, i_loc] <- mask2d[TC2*ci+i, 128*jt+j].
    """
    uniq = {}
    tiles = []
    plan = []
    for ci in range(CI):
        row = []
        for jt in range(JT):
            blk = mask2d[TC2 * ci:TC2 * (ci + 1), 128 * jt:128 * (jt + 1)]
            if blk.all():
                row.append(-1)
            elif not blk.any():
                row.append(None)
            else:
                bias = np.where(blk.T, 0.0, np.float32(MASKED_BIAS)).astype(np.float32)
                key = bias.tobytes()
                if key not in uniq:
                    uniq[key] = len(tiles)
                    tiles.append(bias)
                row.append(uniq[key])
        plan.append(row)
    if not tiles:  # keep the DRAM tensor non-empty
        tiles.append(np.zeros((128, TC2), np.float32))
    return plan, np.stack(tiles)


def _rope_tables():
    inv_freq = 1.0 / (THETA ** (np.arange(0, D, 2, dtype=np.float64) / D))
    freqs = np.outer(inv_freq, np.arange(T, dtype=np.float64))  # [64, T]
    cosI = np.repeat(np.cos(freqs), 2, axis=0).astype(np.float32)  # [128, T]
    sinI = np.repeat(np.sin(freqs), 2, axis=0).astype(np.float32)
    # rot = psignT.T @ x : rot[2i] = -x[2i+1], rot[2i+1] = x[2i]
    psignT = np.zeros((D, D), np.float32)
    for i in range(D // 2):
        psignT[2 * i + 1, 2 * i] = -1.0
        psignT[2 * i, 2 * i + 1] = 1.0
    return cosI, sinI, psignT


def _phase1(nc, tc, qkv_tensors, xT_r, cos_sb, sin_sb):
    qT, kT, vt, wq_h, wk_h, wv_sb, psg_sb = qkv_tensors  # w/cos/sin in wpool
    with tc.tile_pool(name="xt", bufs=2) as xp, \
         tc.tile_pool(name="p1t", bufs=1) as p1, \
         tc.tile_pool(name="ps1", bufs=1, space="PSUM") as pp:
        for tcn in range(NTC1):
            ts = tcn * TC1           # position in flattened BT
            tp = ts % T              # rope position (restarts per batch)
            xt = xp.tile([128, KT, TC1], BF16, tag="xt")
            nparts = 4 if tcn == 0 else 2
            step = KT // nparts
            for q_ in range(nparts):
                nc.sync.dma_start(xt[:, q_ * step:(q_ + 1) * step, :],
                                  xT_r[:, q_ * step:(q_ + 1) * step,
                                       ts:ts + TC1])
            for dst, w_h in ((qT, wq_h), (kT, wk_h)):
                for h in range(HPC):
                    ps = pp.tile([D, TC1], F32, tag="proj", bufs=4)
                    for cc in range(KT):
                        nc.tensor.matmul(
                            ps[:], w_h[h][:, cc, :], xt[:, cc, :],
                            start=(cc == 0), stop=(cc == KT - 1))
                    praw = p1.tile([D, TC1], BF16, tag="praw", bufs=3)
                    nc.vector.tensor_copy(praw[:], ps[:])
                    rot = pp.tile([D, TC1], F32, tag="rot", bufs=2)
                    nc.tensor.matmul(rot[:], psg_sb[:], praw[:],
                                     start=True, stop=True)
                    t1 = p1.tile([D, TC1], BF16, tag="t1", bufs=2)
                    nc.vector.tensor_mul(t1[:], praw[:], cos_sb[:, tp:tp + TC1])
                    t2 = p1.tile([D, TC1], BF16, tag="t2", bufs=2)
                    nc.vector.tensor_mul(t2[:], rot[:], sin_sb[:, tp:tp + TC1])
                    nc.vector.tensor_add(dst[h][:, ts:ts + TC1], t1[:], t2[:])
            # v projection: out [t, d] per 128-row t-tile
            for tt in range(TC1 // 128):
                jt = ts // 128 + tt
                ps = pp.tile([128, HPC * D], F32, tag="vproj", bufs=2)
                for cc in range(KT):
                    nc.tensor.matmul(
                        ps[:], xt[:, cc, tt * 128:(tt + 1) * 128],
                        wv_sb[:, cc, :],
                        start=(cc == 0), stop=(cc == KT - 1))
                nc.vector.tensor_copy(vt[jt][:], ps[:])


def _attn_out(nc, tc, plan, bias_sb, qT, kT, vt, ones_sb,
              a2a_in, a2a_out, wop, wo_sb, y):
    """Phase 2 (attention per head + a2a) and phase 3 (normalize + wo),
    emission-interleaved so the second collective hides under pass-A
    compute: each collective's receive-side loads go out right after its
    trigger (ahead of later sync-queue work), and the k=0 normalize chain
    is emitted mid-head-1 so scaled a2a0 data is ready the moment the PE
    drains head 1."""
    with tc.tile_pool(name="p3", bufs=1) as aop:
        ao = {}      # (k, s) -> raw/scaled output tile
        recb = {}    # k -> bf16 reciprocal denominators [8, TC2]
        yA = [aop.tile([128, TC2], F32, name=f"yA{i}") for i in range(16)]

        def emit_recv(k):
            # receive-side DMAs for collective k, right behind its trigger
            for s in range(N_CORES):
                t_ = aop.tile([128, TC2], BF16, name=f"ao{k}_{s}")
                nc.sync.dma_start(t_[:], a2a_out[k][s, 0:D, :])
                ao[(k, s)] = t_
            den = aop.tile([N_CORES, TC2], BF16, name=f"den{k}")
            for s in range(N_CORES):
                nc.sync.dma_start(den[s:s + 1, :], a2a_out[k][s, D:D + 1, :])
            ao[(k, 'den')] = den

        def emit_norm(k):
            # reciprocal of the softmax denominators + in-place scaling
            den = ao[(k, 'den')]
            denf = aop.tile([N_CORES, TC2], F32, name=f"denf{k}")
            nc.vector.tensor_copy(denf[:], den[:])
            rec = aop.tile([N_CORES, TC2], F32, name=f"rec{k}")
            nc.vector.reciprocal(rec[:], denf[:])
            rb16 = aop.tile([N_CORES, TC2], BF16, name=f"recb{k}")
            nc.vector.tensor_copy(rb16[:], rec[:])
            recb[k] = rb16
            for s in range(N_CORES):
                # partition_broadcast needs its source at partition 0
                rec1 = aop.tile([1, TC2], BF16, tag="rstage", bufs=2)
                nc.sync.dma_start(rec1[:], rb16[s:s + 1, :])
                rb = aop.tile([128, TC2], BF16, tag="rb", bufs=2)
                nc.gpsimd.partition_broadcast(rb[:], rec1[:])
                nc.vector.tensor_mul(ao[(k, s)][:], ao[(k, s)][:], rb[:])

        _phase2(nc, tc, plan, bias_sb, qT, kT, vt, ones_sb, a2a_in, a2a_out,
                emit_recv, emit_norm)

        with tc.tile_pool(name="ps3", bufs=1, space="PSUM") as pp:
            emit_norm(1)
            for k in range(HPC):
                for cj in range(C // TC2):
                    for tt in range(TSL // 128):
                        idx = cj * (TSL // 128) + tt
                        yp = pp.tile([128, TC2], F32, tag="y", bufs=4)
                        for s in range(N_CORES):
                            nc.tensor.matmul(
                                yp[:], ao[(k, s)][:, tt * 128:(tt + 1) * 128],
                                wo_sb[:, s * HPC + k, cj, :],
                                start=(s == 0), stop=(s == N_CORES - 1))
                        if k == 0:
                            # scalar engine is idle here; keep vector free
                            # for the k=1 normalize chain
                            nc.scalar.copy(yA[idx][:], yp[:])
                        else:
                            ysb = wop.tile([128, TC2], F32, tag="ysb", bufs=3)
                            nc.vector.tensor_add(ysb[:], yp[:], yA[idx][:])
                            nc.sync.dma_start(
                                y[tt * 128:(tt + 1) * 128,
                                  cj * TC2:(cj + 1) * TC2],
                                ysb[:])


def _phase2(nc, tc, plan, bias_sb, qT, kT, vt, ones_sb,
            a2a_in, a2a_out, emit_recv, emit_norm):
    with tc.tile_pool(name="p2t", bufs=1) as p2, \
         tc.tile_pool(name="ps2", bufs=1, space="PSUM") as pp:
        for h in range(HPC):
            for b in range(B):
                for ci in range(CI):
                    if h == 1 and b == 1 and ci == 2:
                        # a2a0 has landed by now: normalize its payload
                        # while the PE is still busy with head 1 so pass A
                        # can start the instant head 1 drains (hiding a2a1)
                        emit_norm(0)
                    gci = b * CI + ci      # global chunk == dest rank
                    live = [(jt, plan[ci][jt]) for jt in range(JT)
                            if plan[ci][jt] is not None]
                    if not live:
                        z = p2.tile([128, TC2], BF16, tag="ot", bufs=3)
                        nc.vector.memset(z[:], 0.0)
                        nc.sync.dma_start(a2a_in[h][gci, 0:D, :], z[:])
                        zd = p2.tile([1, TC2], BF16, tag="rps", bufs=2)
                        nc.vector.memset(zd[:], 1.0)
                        nc.sync.dma_start(a2a_in[h][gci, D:D + 1, :], zd[:])
                        continue
                    outp = pp.tile([D, TC2], F32, tag="outT", bufs=3)
                    rp = pp.tile([1, TC2], F32, tag="r", bufs=2)
                    i0 = b * T + ci * TC2
                    qs = qT[h][:, i0:i0 + TC2]
                    for idx, (jt, code) in enumerate(live):
                        jv = (b * T) // 128 + jt
                        sc = pp.tile([128, TC2], F32, tag="sc", bufs=3)
                        nc.tensor.matmul(
                            sc[:],
                            kT[h][:, b * T + jt * 128:b * T + (jt + 1) * 128],
                            qs, start=True, stop=True)
                        if code >= 0:
                            mt = p2.tile([128, TC2], F32, tag="mt", bufs=2)
                            nc.vector.tensor_add(mt[:], sc[:],
                                                 bias_sb[:, code, :])
                            src = mt
                        else:
                            src = sc
                        pt = p2.tile([128, TC2], BF16, tag="pt", bufs=4)
                        nc.scalar.activation(
                            pt[:], src[:], mybir.ActivationFunctionType.Exp,
                            bias=0.0, scale=float(SCALE))
                        nc.tensor.matmul(
                            outp[:], vt[jv][:, h * D:(h + 1) * D], pt[:],
                            start=(idx == 0), stop=(idx == len(live) - 1))
                        nc.tensor.matmul(
                            rp[:], ones_sb[:], pt[:],
                            start=(idx == 0), stop=(idx == len(live) - 1))
                    # ship raw output + denominator; normalize after the a2a
                    ot = p2.tile([128, TC2], BF16, tag="ot", bufs=3)
                    nc.vector.tensor_copy(ot[:], outp[:])
                    nc.sync.dma_start(a2a_in[h][gci, 0:D, :], ot[:])
                    rps = p2.tile([1, TC2], BF16, tag="rps", bufs=2)
                    nc.vector.tensor_copy(rps[:], rp[:])
                    nc.sync.dma_start(a2a_in[h][gci, D:D + 1, :], rps[:])
            # this head's comm overlaps the next head's compute
            nc.gpsimd.collective_compute(
                "AllToAll", mybir.AluOpType.bypass,
                replica_groups=[list(range(N_CORES))],
                ins=[a2a_in[h].opt()], outs=[a2a_out[h].opt()])
            # queue the receive-side loads NOW so they sit ahead of the
            # next head's output DMAs on the sync queue
            emit_recv(h)


def _build(plan, n_bias):
    nc = bacc.Bacc("TRN2", num_devices=N_CORES)

    # weights arrive host-packed in the exact SBUF tile layouts so every
    # DMA row is 4-16 KB contiguous (256-byte rows gated startup before)
    xT = nc.dram_tensor("xT", [C, BT], BF16, kind="ExternalInput")
    wq = nc.dram_tensor("wq", [128, HPC, KT, D], BF16, kind="ExternalInput")
    wk = nc.dram_tensor("wk", [128, HPC, KT, D], BF16, kind="ExternalInput")
    wv = nc.dram_tensor("wv", [128, KT, HPC * D], BF16, kind="ExternalInput")
    wo = nc.dram_tensor("wo", [128, KT, C // TC2, TC2], BF16,
                        kind="ExternalInput")
    cos_d = nc.dram_tensor("cos", [D, T], BF16, kind="ExternalInput")
    sin_d = nc.dram_tensor("sin", [D, T], BF16, kind="ExternalInput")
    psg_d = nc.dram_tensor("psg", [D, D], BF16, kind="ExternalInput")
    ones_d = nc.dram_tensor("ones", [128, 1], BF16, kind="ExternalInput")
    bias_d = nc.dram_tensor("bias", [n_bias, 128, TC2], F32, kind="ExternalInput")
    y = nc.dram_tensor("y", [TSL, C], F32, kind="ExternalOutput")

    xT_r = xT.rearrange("(n p) t -> p n t", p=128)

    with tile.TileContext(nc) as tc:
        with tc.tile_pool(name="const", bufs=1) as cpool, \
             tc.tile_pool(name="dram", bufs=1, space="DRAM") as dram:

            a2a_in = [dram.tile([N_CORES, D + 1, TC2], BF16, name=f"a2ai{h}")
                      for h in range(HPC)]
            a2a_out = [dram.tile([N_CORES, D + 1, TC2], BF16, name=f"a2ao{h}")
                       for h in range(HPC)]

            with tc.tile_pool(name="qkv", bufs=1) as qkv:
                qT = [qkv.tile([D, BT], BF16, name=f"qT{h}") for h in range(HPC)]
                kT = [qkv.tile([D, BT], BF16, name=f"kT{h}") for h in range(HPC)]
                vt = [qkv.tile([128, HPC * D], BF16, name=f"v{j}")
                      for j in range(BT // 128)]

                with tc.tile_pool(name="wp", bufs=1) as wp:
                    # startup: spread descriptor generation across engine
                    # queues -- sync only carries the xt chunks so the first
                    # matmul's data is in flight immediately.
                    wq_h = []
                    for h in range(HPC):
                        w_ = wp.tile([128, KT, D], BF16, name=f"wqh{h}")
                        nc.scalar.dma_start(w_[:], wq[:, h, :, :])
                        wq_h.append(w_)
                    psg_sb = cpool.tile([D, D], BF16)
                    nc.gpsimd.dma_start(psg_sb[:], psg_d[:])
                    ones_sb = cpool.tile([128, 1], BF16)
                    nc.gpsimd.dma_start(ones_sb[:], ones_d[:])
                    warm = cpool.tile([128, 1], F32)
                    nc.scalar.activation(warm[:], ones_sb[:],
                                         mybir.ActivationFunctionType.Exp,
                                         bias=0.0, scale=1.0)
                    warm2 = cpool.tile([128, 1], BF16)
                    nc.gpsimd.partition_broadcast(warm2[:], ones_sb[0:1, :])
                    wk_h = []
                    for h in range(HPC):
                        w_ = wp.tile([128, KT, D], BF16, name=f"wkh{h}")
                        nc.gpsimd.dma_start(w_[:], wk[:, h, :, :])
                        wk_h.append(w_)
                    cos_sb = wp.tile([D, T], BF16)
                    nc.gpsimd.dma_start(cos_sb[:], cos_d[:])
                    sin_sb = wp.tile([D, T], BF16)
                    nc.gpsimd.dma_start(sin_sb[:], sin_d[:])
                    wv_sb = wp.tile([128, KT, HPC * D], BF16)
                    nc.scalar.dma_start(wv_sb[:], wv[:])

                    _phase1(nc, tc, (qT, kT, vt, wq_h, wk_h, wv_sb, psg_sb),
                            xT_r, cos_sb, sin_sb)

                # wo pool opens as soon as the phase-1 weights are freed; the
                # full wo (8 MB bf16) + bias prefetch on the idle gpsimd queue
                # so they land under phase-2 compute
                with tc.tile_pool(name="wo", bufs=1) as wop:
                    bias_sb = wop.tile([128, n_bias, TC2], F32)
                    nc.gpsimd.dma_start(bias_sb[:],
                                        bias_d.rearrange("u p m -> p u m"))
                    wo_sb = wop.tile([128, KT, C // TC2, TC2], BF16)
                    for q_ in range(8):
                        nc.gpsimd.dma_start(
                            wo_sb[:, 2 * q_:2 * (q_ + 1), :, :],
                            wo[:, 2 * q_:2 * (q_ + 1), :, :])
                    _attn_out(nc, tc, plan, bias_sb, qT, kT, vt,
                              ones_sb, a2a_in, a2a_out, wop, wo_sb, y)

    nc.finalize()
    return nc


_cache = {}


def _get_kernel(mask2d):
    key = mask2d.tobytes()
    if key not in _cache:
        plan, bias_tiles = _mask_plan(mask2d)
        nc = _build(plan, bias_tiles.shape[0])
        _cache[key] = (nc, bias_tiles)
    return _cache[key]


def kernel(x, mask, wq, wk, wv, wo, _trace=False):
    x = np.asarray(x)
    mask2d = np.asarray(mask).reshape(T, T).astype(bool)
    nc, bias_tiles = _get_kernel(mask2d)

    cosI, sinI, psignT = _rope_tables()
    xT_full = _bf16(np.asarray(x).reshape(BT, C).T)

    def pack_qk(ws):     # [C, HPC*D] -> [128, HPC, KT, D]
        return _bf16(np.asarray(ws).reshape(KT, 128, HPC, D)
                     .transpose(1, 2, 0, 3))

    def pack_v(ws):      # [C, HPC*D] -> [128, KT, HPC*D]
        return _bf16(np.asarray(ws).reshape(KT, 128, HPC * D)
                     .transpose(1, 0, 2))

    wo_p = _bf16(np.asarray(wo).reshape(KT, 128, C // TC2, TC2)
                 .transpose(1, 0, 2, 3))
    common = {
        "cos": _bf16(cosI), "sin": _bf16(sinI), "psg": _bf16(psignT),
        "ones": np.ones((128, 1), NPBF),
        "bias": bias_tiles, "wo": wo_p, "xT": xT_full,
    }
    in_maps = []
    for c in range(N_CORES):
        sl = slice(c * HPC * D, (c + 1) * HPC * D)
        in_maps.append({
            "wq": pack_qk(np.asarray(wq)[:, sl]),
            "wk": pack_qk(np.asarray(wk)[:, sl]),
            "wv": pack_v(np.asarray(wv)[:, sl]),
            **common,
        })

    r = run_bass_kernel_spmd(nc, in_maps, core_ids=list(range(N_CORES)),
                             trace=_trace)
    out = np.empty((BT, C), np.float32)
    for c in range(N_CORES):
        out[c * TSL:(c + 1) * TSL, :] = r.results[c]["y"]
    if _trace:
        kernel.last_results = r
    return out.reshape(B, T, C)
